# revision 32
# baseline (speedup 1.0000x reference)
"""Trainium2 Bass kernel for nn_Decoder (attention GRU decoder + classifier).

Key algebraic simplification: the additive-attention logits are
  s[b,t] = score_x[b,t] + (h @ Wa_h)[b]
and softmax over t is invariant to the per-b constant shift, so the attention
weights -- and therefore ctx and gi = ctx @ W_ih.T -- are the same for all 22
steps.  The recurrence reduces to gh = h @ W_hh.T per step.

Sharding: pure data-parallel over batch, 16 rows per core, no collectives.
"""

import sys

for _p in ("/root/.axon_site",):
    if _p not in sys.path:
        sys.path.insert(0, _p)

import numpy as np

import concourse.bass as bass
import concourse.bacc as bacc
import concourse.mybir as mybir
from concourse import tile
from concourse.bass_utils import run_bass_kernel_spmd
from concourse.masks import make_identity

dt = mybir.dt
AF = mybir.ActivationFunctionType
ALU = mybir.AluOpType

N_CORES = 8
B, T, D, H, C = 128, 512, 512, 512, 4367
S = 22
BL = B // N_CORES  # 16 batch rows per core
TC, DC, HC = T // 128, D // 128, H // 128
G3 = 3 * H  # 1536

MMDT = dt.float16  # matmul operand dtype (1 cyc/row on PE, fp32 PSUM accum)


def _build():
    nc = bacc.Bacc("TRN2", target_bir_lowering=False, debug=False,
                   num_devices=N_CORES)

    x_d = nc.dram_tensor("x", [BL, T, D], dt.float16, kind="ExternalInput").ap()
    wproj_d = nc.dram_tensor("W_proj", [H, D], dt.float16, kind="ExternalInput").ap()
    bproj_d = nc.dram_tensor("b_proj", [H], dt.float32, kind="ExternalInput").ap()
    walign_d = nc.dram_tensor("W_align", [1, H + D], dt.float16, kind="ExternalInput").ap()
    wih_d = nc.dram_tensor("W_ih", [G3, D], dt.float16, kind="ExternalInput").ap()
    bih_d = nc.dram_tensor("b_ih", [G3], dt.float32, kind="ExternalInput").ap()
    whh_d = nc.dram_tensor("W_hh", [G3, H], dt.float16, kind="ExternalInput").ap()
    bhh_d = nc.dram_tensor("b_hh", [G3], dt.float32, kind="ExternalInput").ap()
    wcls_d = nc.dram_tensor("W_cls", [C, H], dt.float16, kind="ExternalInput").ap()
    bcls_d = nc.dram_tensor("b_cls", [C], dt.float32, kind="ExternalInput").ap()
    y_d = nc.dram_tensor("y", [BL, S, C], dt.float32, kind="ExternalOutput").ap()
    y_flat = y_d.rearrange("b s c -> (b s) c")  # row b*S+s

    with tile.TileContext(nc) as tc:
        _emit(nc, tc, x_d, wproj_d, bproj_d, walign_d, wih_d, bih_d,
              whh_d, bhh_d, wcls_d, bcls_d, y_flat)
    nc.compile()
    return nc


def _emit(nc, tc, x_d, wproj_d, bproj_d, walign_d, wih_d, bih_d,
          whh_d, bhh_d, wcls_d, bcls_d, y_flat):
    from contextlib import ExitStack
    ctx_stack = ExitStack()
    with ctx_stack:
        wts = ctx_stack.enter_context(tc.tile_pool(name="wts", bufs=1))
        stage = ctx_stack.enter_context(tc.tile_pool(name="stage", bufs=2))
        work = ctx_stack.enter_context(tc.tile_pool(name="work", bufs=2))
        ps_w = ctx_stack.enter_context(
            tc.tile_pool(name="ps_w", bufs=1, space="PSUM"))
        ps_main = ctx_stack.enter_context(
            tc.tile_pool(name="ps_main", bufs=3, space="PSUM"))
        ps_ctx = ctx_stack.enter_context(
            tc.tile_pool(name="ps_ctx", bufs=1, space="PSUM"))
        ps_htr = ctx_stack.enter_context(
            tc.tile_pool(name="ps_htr", bufs=2, space="PSUM"))

        # ---- constants ----
        ident = wts.tile([128, 128], dt.float32)
        make_identity(nc, ident[:])
        ident_h = wts.tile([128, 128], MMDT)
        nc.vector.tensor_copy(ident_h[:], ident[:])
        ident16_r = wts.tile([16, 16], MMDT)
        nc.vector.tensor_copy(ident16_r[:], ident[:16, :16])
        ones_mat = wts.tile([128, 128], MMDT)
        nc.vector.memset(ones_mat[:], 1.0)
        ones1_r = wts.tile([1, 128], MMDT)
        nc.vector.memset(ones1_r[:], 1.0)

        # Wa_x broadcast to all 128 partitions via PE outer product
        wax_row = wts.tile([1, D], MMDT)
        nc.sync.dma_start(wax_row[:], walign_d[:, :D])
        wax_ps = ps_main.tile([128, D], dt.float32, tag="mm")
        nc.tensor.matmul(wax_ps[:], ones1_r[:], wax_row[:], start=True, stop=True)
        wax = wts.tile([128, D], MMDT)
        nc.vector.tensor_copy(wax[:], wax_ps[:])

        # ---- bias rows ----
        brow_ih = wts.tile([1, G3], dt.float32)
        nc.sync.dma_start(brow_ih[:], bih_d[None, :])
        brow_hh = wts.tile([1, G3], dt.float32)
        nc.sync.dma_start(brow_hh[:], bhh_d[None, :])
        brow_proj_f = wts.tile([1, H], dt.float32)
        nc.sync.dma_start(brow_proj_f[:], bproj_d[None, :])
        brow_proj_r = wts.tile([1, H], MMDT)
        nc.vector.tensor_copy(brow_proj_r[:], brow_proj_f[:])
        brow_cls_f = wts.tile([1, C], dt.float32)
        nc.sync.dma_start(brow_cls_f[:], bcls_d[None, :])
        brow_cls_r = wts.tile([1, C], MMDT)
        nc.vector.tensor_copy(brow_cls_r[:], brow_cls_f[:])
        # r,z gates: b_ih + b_hh fused; n gate: b_ih only (b_hh_n stays inside r*(.))
        bias_rz_r = wts.tile([1, 2 * H], MMDT)
        nc.vector.tensor_tensor(out=bias_rz_r[:], in0=brow_ih[:, :2 * H],
                                in1=brow_hh[:, :2 * H], op=ALU.add)
        bias_n_r = wts.tile([1, H], MMDT)
        nc.vector.tensor_copy(bias_n_r[:], brow_ih[:, 2 * H:])
        bhh_n_r = wts.tile([1, H], MMDT)
        nc.vector.tensor_copy(bhh_n_r[:], brow_hh[:, 2 * H:])

        # ---- weight transposes:  W[out,in] -> WT[in-part, out-free] ----
        def load_transposed(w_dram, n_out, name, copy_engine=nc.vector):
            """w_dram [n_out, 512] -> tile [128, 4, n_out] (MMDT)."""
            wt_t = wts.tile([128, DC, n_out], MMDT, tag=name)
            n_chunks = (n_out + 127) // 128
            for ci in range(n_chunks):
                rc = min(128, n_out - ci * 128)
                nat = stage.tile([128, 512], MMDT, tag="wnat")
                nc.sync.dma_start(nat[:rc, :], w_dram[ci * 128: ci * 128 + rc, :])
                for dc_i in range(DC):
                    pt = ps_w.tile([128, 128], MMDT, tag="wtr")
                    nc.tensor.matmul(pt[:, :rc],
                                     nat[:rc, dc_i * 128:(dc_i + 1) * 128],
                                     ident_h[:rc, :rc], is_transpose=True)
                    if copy_engine is nc.vector:
                        copy_engine.tensor_copy(
                            wt_t[:, dc_i, ci * 128: ci * 128 + rc], pt[:, :rc])
                    else:
                        copy_engine.copy(
                            wt_t[:, dc_i, ci * 128: ci * 128 + rc], pt[:, :rc])
            return wt_t

        wihT = load_transposed(wih_d, G3, "wihT")
        wprojT = load_transposed(wproj_d, H, "wprojT")
        whhT = load_transposed(whh_d, G3, "whhT")

        # W_cls is transposed lazily: chunks are emitted inside the recurrence
        # loop as PE filler work (keeps HAM warm during gate stalls).
        wclsT = wts.tile([128, DC, C], MMDT, tag="wclsT")
        wcls_chunks = list(range((C + 127) // 128))

        def emit_wcls_chunk(ci):
            rc = min(128, C - ci * 128)
            nat = stage.tile([128, 512], MMDT, tag="wnat")
            nc.sync.dma_start(nat[:rc, :], wcls_d[ci * 128: ci * 128 + rc, :])
            for dc_i in range(DC):
                pt = ps_w.tile([128, 128], MMDT, tag="wtr")
                nc.tensor.matmul(pt[:, :rc],
                                 nat[:rc, dc_i * 128:(dc_i + 1) * 128],
                                 ident_h[:rc, :rc], is_transpose=True)
                nc.vector.tensor_copy(
                    wclsT[:, dc_i, ci * 128: ci * 128 + rc], pt[:, :rc])

        # ---- attention phase (per-b streaming) ----
        sums_bc = wts.tile([128, BL], dt.float32)
        ctx_acc = ps_ctx.tile([128, DC, BL], dt.float32, tag="ctx")
        ctx_ps = [ctx_acc[:, dc_i, :] for dc_i in range(DC)]
        xpool = tc.tile_pool(name="xp", bufs=3)
        with xpool as xp:
            for b in range(BL):
                xb_h = xp.tile([128, TC, D], MMDT, tag="xb_h")
                nc.sync.dma_start(
                    xb_h[:],
                    x_d[b].rearrange("(tc tp) d -> tp tc d", tp=128))
                s_b = work.tile([128, TC], dt.float32, tag="s_b")
                for tc_i in range(TC):
                    prod = work.tile([128, D], MMDT, tag="prod")
                    nc.vector.tensor_tensor(out=prod[:], in0=xb_h[:, tc_i, :],
                                            in1=wax[:], op=ALU.mult)
                    junk = work.tile([128, D], MMDT, tag="junk")
                    nc.scalar.activation(junk[:], prod[:], AF.Copy,
                                         accum_out=s_b[:, tc_i:tc_i + 1])
                e_b = work.tile([128, TC], MMDT, tag="e_b")
                nc.scalar.activation(e_b[:], s_b[:], AF.Exp)
                sum_ps = ps_w.tile([128, TC], dt.float32, tag="esum")
                nc.tensor.matmul(sum_ps[:], ones_mat[:], e_b[:],
                                 start=True, stop=True)
                nc.vector.tensor_reduce(
                    out=sums_bc[:, b:b + 1], in_=sum_ps[:],
                    axis=mybir.AxisListType.X, op=ALU.add)
                # One accumulation group per bank across the whole phase:
                # start=True zeroes the entire 2KB zero-region, so only the
                # very first matmul into the bank may carry it.
                for dc_i in range(DC):
                    for tc_i in range(TC):
                        nc.tensor.matmul(
                            ctx_ps[dc_i][:, b:b + 1],
                            xb_h[:, tc_i, dc_i * 128:(dc_i + 1) * 128],
                            e_b[:, tc_i:tc_i + 1],
                            start=(b == 0 and dc_i == 0 and tc_i == 0),
                            stop=(b == BL - 1 and dc_i == DC - 1
                                  and tc_i == TC - 1))

        recip_bc = wts.tile([128, BL], dt.float32)
        nc.vector.reciprocal(recip_bc[:], sums_bc[:])
        ctxT = wts.tile([128, DC, BL], MMDT)
        for dc_i in range(DC):
            nc.vector.tensor_tensor(out=ctxT[:, dc_i, :], in0=ctx_ps[dc_i][:],
                                    in1=recip_bc[:], op=ALU.mult)

        # ---- x last frame -> h0 ----
        xlast = wts.tile([BL, D], MMDT)
        nc.sync.dma_start(xlast[:], x_d[:, T - 1, :])
        xlastT = wts.tile([128, DC, BL], MMDT)
        for dc_i in range(DC):
            pt = ps_w.tile([128, BL], MMDT, tag="wtr")
            nc.tensor.matmul(pt[:], xlast[:, dc_i * 128:(dc_i + 1) * 128],
                             ident_h[:BL, :BL], is_transpose=True)
            nc.vector.tensor_copy(xlastT[:, dc_i, :], pt[:])

        # ---- gi_full = ctx @ W_ih.T + biases (loop-invariant) ----
        gi_full = wts.tile([BL, G3], MMDT)
        for g in range(3):
            pt = ps_main.tile([BL, H], dt.float32, tag="mm")
            for dc_i in range(DC):
                nc.tensor.matmul(pt[:], ctxT[:, dc_i, :],
                                 wihT[:, dc_i, g * H:(g + 1) * H],
                                 start=(dc_i == 0), stop=False)
            brow = bias_rz_r[:, g * H:(g + 1) * H] if g < 2 else bias_n_r[:]
            nc.tensor.matmul(pt[:], ones1_r[:, :BL], brow, start=False, stop=True)
            nc.vector.tensor_copy(gi_full[:, g * H:(g + 1) * H], pt[:])
            if g == 2:
                gi_n_f32 = wts.tile([BL, H], dt.float32)
                nc.vector.tensor_copy(gi_n_f32[:], pt[:])

        # ---- h0 = x_last @ W_proj.T + b_proj ----
        h0_ps = ps_main.tile([BL, H], dt.float32, tag="mm")
        for dc_i in range(DC):
            nc.tensor.matmul(h0_ps[:], xlastT[:, dc_i, :], wprojT[:, dc_i, :],
                             start=(dc_i == 0), stop=False)
        nc.tensor.matmul(h0_ps[:], ones1_r[:, :BL], brow_proj_r[:],
                         start=False, stop=True)
        h_row = work.tile([BL, H], dt.float32, tag="h_row")
        nc.vector.tensor_copy(h_row[:], h0_ps[:])

        # hsT: [128, hc, b, s] fp32r; merged (b s) view is the classifier lhsT
        hsT = wts.tile([128, HC, BL, S], MMDT)

        def transpose_h(h_row_t, step):
            """h_row [16,512] -> hT [128, hc, 16]; also into hsT at `step`."""
            hT = work.tile([128, HC, BL], MMDT, tag="hT")
            for hc_i in range(HC):
                pt = ps_htr.tile([128, BL], dt.float32, tag="htr")
                nc.tensor.matmul(pt[:], h_row_t[:, hc_i * 128:(hc_i + 1) * 128],
                                 ident[:BL, :BL], is_transpose=True)
                nc.vector.tensor_copy(hT[:, hc_i, :], pt[:])
                if step is not None:
                    nc.scalar.copy(hsT[:, hc_i, :, step], pt[:])
            return hT

        hT = transpose_h(h_row, None)

        # ---- recurrence ----
        for s in range(S):
            gh = []
            for _gi in range(3):
                gh_t = ps_main.tile([BL, H], dt.float32, tag="mm")
                gh.append(gh_t)
            for g in range(3):
                for dc_i in range(DC):
                    nc.tensor.matmul(gh[g][:], hT[:, dc_i, :],
                                     whhT[:, dc_i, g * H:(g + 1) * H],
                                     start=(dc_i == 0), stop=False)
                if g < 2:
                    nc.tensor.matmul(gh[g][:], ident16_r[:],
                                     gi_full[:, g * H:(g + 1) * H],
                                     start=False, stop=True)
                else:
                    nc.tensor.matmul(gh[g][:], ones1_r[:, :BL], bhh_n_r[:],
                                     start=False, stop=True)
            r_sb = work.tile([BL, H], dt.float32, tag="r_sb")
            nc.scalar.activation(r_sb[:], gh[0][:], AF.Sigmoid)
            z_sb = work.tile([BL, H], dt.float32, tag="z_sb")
            nc.scalar.activation(z_sb[:], gh[1][:], AF.Sigmoid)
            # omz = 1 - z and zh = z*h run in parallel with the n-gate chain
            omz = work.tile([BL, H], dt.float32, tag="omz")
            nc.scalar.activation(omz[:], z_sb[:], AF.Identity, scale=-1.0, bias=1.0)
            zh = work.tile([BL, H], dt.float32, tag="zh")
            nc.vector.tensor_tensor(out=zh[:], in0=z_sb[:], in1=h_row[:],
                                    op=ALU.mult)
            rhn = work.tile([BL, H], dt.float32, tag="rhn")
            nc.vector.tensor_tensor(out=rhn[:], in0=r_sb[:], in1=gh[2][:],
                                    op=ALU.mult)
            pre_n = work.tile([BL, H], dt.float32, tag="pre_n")
            nc.vector.tensor_tensor(out=pre_n[:], in0=rhn[:],
                                    in1=gi_n_f32[:], op=ALU.add)
            n_sb = work.tile([BL, H], dt.float32, tag="n_sb")
            nc.scalar.activation(n_sb[:], pre_n[:], AF.Tanh)
            # h_new = (1-z)*n + z*h
            on_sb = work.tile([BL, H], dt.float32, tag="on_sb")
            nc.vector.tensor_tensor(out=on_sb[:], in0=omz[:], in1=n_sb[:],
                                    op=ALU.mult)
            h_row = work.tile([BL, H], dt.float32, tag="h_row")
            nc.vector.tensor_tensor(out=h_row[:], in0=on_sb[:], in1=zh[:],
                                    op=ALU.add)
            hT = transpose_h(h_row, s)
            # PE filler: transpose two W_cls chunks per step (keeps HAM warm)
            for _ in range(2):
                if wcls_chunks:
                    emit_wcls_chunk(wcls_chunks.pop(0))

        # ---- classifier: logits[(b s), c] = hs @ W_cls.T + b_cls ----
        while wcls_chunks:
            emit_wcls_chunk(wcls_chunks.pop(0))
        hsT_flat = hsT[:].rearrange("p hc b s -> p hc (b s)")
        BS = BL * S  # 352
        m_chunks = [(0, 128), (128, 128), (256, BS - 256)]
        n_starts = list(range(0, C, 512))
        for mi, (m0, mc_sz) in enumerate(m_chunks):
            for n0 in n_starts:
                n_sz = min(512, C - n0)
                pt = ps_main.tile([128, 512], dt.float32, tag="mm")
                for kc in range(HC):
                    nc.tensor.matmul(pt[:mc_sz, :n_sz],
                                     hsT_flat[:, kc, m0:m0 + mc_sz],
                                     wclsT[:, kc, n0:n0 + n_sz],
                                     start=(kc == 0), stop=False)
                nc.tensor.matmul(pt[:mc_sz, :n_sz], ones1_r[:, :mc_sz],
                                 brow_cls_r[:, n0:n0 + n_sz],
                                 start=False, stop=True)
                ot = work.tile([128, 512], dt.float32, tag="cls_out")
                if (n0 // 512) % 2 == 0:
                    nc.vector.tensor_copy(ot[:mc_sz, :n_sz], pt[:mc_sz, :n_sz])
                else:
                    nc.scalar.copy(ot[:mc_sz, :n_sz], pt[:mc_sz, :n_sz])
                nc.sync.dma_start(y_flat[m0:m0 + mc_sz, n0:n0 + n_sz],
                                  ot[:mc_sz, :n_sz])


_NC_CACHE = None


def _get_nc():
    global _NC_CACHE
    if _NC_CACHE is None:
        _NC_CACHE = _build()
    return _NC_CACHE


def kernel(**inputs):
    x = np.ascontiguousarray(inputs["x"], dtype=np.float16)
    n_steps = int(inputs["n_steps"])
    assert n_steps == S, f"kernel compiled for n_steps={S}, got {n_steps}"
    assert x.shape == (B, T, D)

    weights = {}
    for k in ("W_proj", "W_align", "W_ih", "W_hh", "W_cls"):
        weights[k] = np.ascontiguousarray(np.asarray(inputs[k]), dtype=np.float16)
    for k in ("b_proj", "b_ih", "b_hh", "b_cls"):
        weights[k] = np.ascontiguousarray(np.asarray(inputs[k]), dtype=np.float32)
    # b_align shifts every logit equally -> softmax-invariant, unused.

    nc = _get_nc()
    in_maps = []
    for i in range(N_CORES):
        m = dict(weights)
        m["x"] = x[i * BL:(i + 1) * BL]
        in_maps.append(m)
    res = run_bass_kernel_spmd(nc, in_maps, list(range(N_CORES)))
    out = np.concatenate([res.results[i]["y"] for i in range(N_CORES)], axis=0)
    return out.astype(np.float32)


if __name__ == "__main__":
    rng = np.random.default_rng(0)
    ins = {
        "x": rng.standard_normal((B, T, D)).astype(np.float32),
        "W_proj": (rng.standard_normal((H, D)) * 0.02).astype(np.float32),
        "b_proj": np.zeros(H, np.float32),
        "W_align": (rng.standard_normal((1, H + D)) * 0.02).astype(np.float32),
        "b_align": np.zeros(1, np.float32),
        "W_ih": (rng.standard_normal((G3, D)) * 0.02).astype(np.float32),
        "b_ih": np.zeros(G3, np.float32),
        "W_hh": (rng.standard_normal((G3, H)) * 0.02).astype(np.float32),
        "b_hh": np.zeros(G3, np.float32),
        "W_cls": (rng.standard_normal((C, H)) * 0.02).astype(np.float32),
        "b_cls": np.zeros(C, np.float32),
        "n_steps": np.int64(S),
    }
    y = kernel(**ins)
    print("out", y.shape, y.dtype, float(np.abs(y).max()))
